# revision 7
# baseline (speedup 1.0000x reference)
"""FM layer (gather + segment_sum + 0.5*(s^2 - sum_sq)) on 8 Trainium2 cores.

Strategy (data parallel over batch rows):
  - core c owns batch rows [512c, 512(c+1)); batch_ids are sorted, so each
    core's nonzeros are a contiguous slice found by searchsorted.
  - host packs the embedding table as [e | e^2] f32 -> 512B rows so one
    512B gather descriptor (line-rate on SDMA) delivers both factors.
  - per core, nonzeros are sorted by (vocab bucket of 32768 rows, 128-row
    batch chunk) and padded per (bucket,chunk) segment to a multiple of 128
    with (idx=0, v=0) entries; segment sizes are equalized across cores so
    a single SPMD program serves all 8 cores.
  - one gpsimd.dma_gather per bucket (int16 indices into a 32768-row table
    slice) pulls all that bucket's rows to SBUF.
  - per 128-nonzero tile: DVE builds W[p,b] = (ramp==lid[p])*v[p], DVE
    scales the e^2 half by v in place, and PE accumulates
    psum[chunk] += W^T @ [e | v*e^2] giving [s | q] per 128-row chunk.
  - epilogue per chunk: out = 0.5*(s*s - q) on DVE, DMA to DRAM.
  - host concatenates the 8 per-core [512, 64] outputs.
"""
import numpy as np

NCORES = 8
VOCAB = 1000000
EMBED = 64
BATCH = 4096
ROWS_PER_CORE = BATCH // NCORES          # 512
CHUNKS = ROWS_PER_CORE // 128            # 4
BUCKET = 32768
NBUCKETS = (VOCAB + BUCKET - 1) // BUCKET  # 31 full + remainder => 31? 1e6/32768 = 30.5 -> 31
D2 = 2 * EMBED                           # 128 packed row elements


def _build_plan(feat_ids, batch_ids):
    """Host-side sharding plan. Returns per-core index/metadata arrays and the
    static tile schedule shared by all cores."""
    core_lo = np.searchsorted(batch_ids, np.arange(NCORES) * ROWS_PER_CORE)
    core_hi = np.searchsorted(batch_ids, (np.arange(NCORES) + 1) * ROWS_PER_CORE)

    per_core = []
    counts = np.zeros((NCORES, NBUCKETS, CHUNKS), np.int64)
    for c in range(NCORES):
        lo, hi = core_lo[c], core_hi[c]
        fid = feat_ids[lo:hi].astype(np.int64)
        bid = batch_ids[lo:hi].astype(np.int64) - c * ROWS_PER_CORE
        bucket = fid // BUCKET
        chunk = bid >> 7
        order = np.lexsort((chunk, bucket))
        fid, bid, bucket, chunk = fid[order], bid[order], bucket[order], chunk[order]
        per_core.append((lo, order, fid, bid, bucket, chunk))
        np.add.at(counts[c], (bucket, chunk), 1)

    pad = (np.ceil(counts.max(axis=0) / 128).astype(np.int64) * 128)  # [NBUCKETS, CHUNKS]
    seg_off = np.zeros((NBUCKETS, CHUNKS), np.int64)
    off = 0
    bucket_span = []  # (start_elem, n_elem) per bucket
    tile_chunk = []   # chunk id per global tile
    for b in range(NBUCKETS):
        b0 = off
        for ch in range(CHUNKS):
            seg_off[b, ch] = off
            tile_chunk.extend([ch] * (pad[b, ch] // 128))
            off += pad[b, ch]
        bucket_span.append((b0, off - b0))
    tot = off
    tiles = tot // 128
    tile_chunk = np.array(tile_chunk, np.int64)

    # first/last tile per chunk (for PSUM start/stop flags)
    first_tile = {}
    last_tile = {}
    for t, ch in enumerate(tile_chunk):
        first_tile.setdefault(int(ch), t)
        last_tile[int(ch)] = t

    return per_core, pad, seg_off, bucket_span, tot, tiles, tile_chunk, first_tile, last_tile


def _build_core_arrays(core_data, pad, seg_off, bucket_span, tot, feature_vals):
    lo, order, fid, bid, bucket, chunk = core_data
    v = feature_vals[lo:lo + len(order)].astype(np.float32)[order]

    # absolute position of every (sorted) nonzero in the padded stream;
    # within each (bucket, chunk) segment entries are consecutive in sorted order
    seg_id = bucket * pad.shape[1] + chunk
    seg_start = seg_off.reshape(-1)
    # index within segment = running count; sorted order makes segments contiguous
    change = np.ones(len(seg_id), bool)
    change[1:] = seg_id[1:] != seg_id[:-1]
    seg_first = np.where(change)[0]
    run = np.arange(len(seg_id)) - np.repeat(seg_first, np.diff(np.append(seg_first, len(seg_id))))
    pos = seg_start[seg_id] + run

    idx_local = np.zeros(tot, np.int64)          # index within 32768-row bucket slice
    lid = np.zeros(tot, np.float32)              # local row id within 128-row chunk
    val = np.zeros(tot, np.float32)
    idx_local[pos] = fid % BUCKET
    lid[pos] = (bid & 127).astype(np.float32)
    val[pos] = v

    # idx16: per-bucket-call wrap-16 layout, replicated across 8 groups of 16
    idx16 = np.zeros((128, tot // 16), np.int16)
    for b0, n in bucket_span:
        if n == 0:
            continue
        blk = idx_local[b0:b0 + n].reshape(n // 16, 16).T.astype(np.int16)  # [16, n/16]
        col0 = b0 // 16
        for g in range(8):
            idx16[g * 16:(g + 1) * 16, col0:col0 + n // 16] = blk

    tiles = tot // 128
    lidf = lid.reshape(tiles, 128).T.copy()      # [128, tiles]
    valf = val.reshape(tiles, 128).T.copy()
    return idx16, lidf, valf


def _strip_redundant_self_waits(nc):
    """Drop waits on an instruction's own engine-proc semaphore when program
    order already guarantees them (threshold <= prior same-engine updates).
    Tile emits these for pool-slot WAW; they force Bacc's one-wait-per-inst
    event-semaphore splitting, which serializes the hot loop."""
    import concourse.mybir as mybir

    proc_of_engine = {
        mybir.EngineType.PE: "PE_",
        mybir.EngineType.DVE: "DVE_",
        mybir.EngineType.Activation: "Activation_",
        mybir.EngineType.Pool: "Pool_",
        mybir.EngineType.SP: "SP_",
    }
    for blk in nc.m.functions[0].blocks:
        counts = {}
        for ins in blk.instructions:
            si = ins.sync_info
            eng = ins.engine
            pref = proc_of_engine.get(eng)
            if si is not None and si.on_wait and type(ins).__name__ == "InstDMAGatherAnt":
                # Keep only the PE slot-WAR leash and the idx-load (DMAHW)
                # wait. The DVE WAR is implied: each MM reads the q-scaled
                # rows, so MM(t) complete => q-scale(t) complete, and the PE
                # wait already covers the slot's MMs. The DMASW lane wait is
                # conservative bookkeeping (queue-0 DMAs are FIFO and sem
                # thresholds are monotonic). One wait avoids Bacc's
                # event-semaphore split, which serializes the pipeline.
                si.on_wait = [w for w in si.on_wait
                              if (w.ant_name or "").startswith(("PE_", "DMAHW"))]
            if si is not None and si.on_wait and pref is not None:
                kept = []
                for w in si.on_wait:
                    name = w.ant_name or ""
                    if (w.wait_mode == "sem-ge-imm"
                            and name.startswith(pref)
                            and w.wait_value <= counts.get(name, 0)):
                        continue
                    kept.append(w)
                if len(kept) != len(si.on_wait):
                    si.on_wait = kept
            if si is not None:
                for u in si.on_update:
                    name = u.ant_name or ""
                    if u.update_mode == "sem-inc" and name.startswith(
                            proc_of_engine.get(eng, "\0")):
                        counts[name] = counts.get(name, 0) + u.update_value


def _build_bass(tot, tiles, bucket_span, tile_chunk, first_tile, last_tile):
    import concourse.bacc as bacc
    import concourse.mybir as mybir
    from concourse.bass import AP
    from concourse.tile import TileContext

    # 2 SWDGE queues: desc-gen for alternating gathers runs on both SWDGE Q7
    # cores concurrently (measured 2x; 4 queues adds nothing - only 2 cores).
    nc = bacc.Bacc(trn_type="TRN2", num_swdge_queues=4)
    table = nc.dram_tensor("table", [VOCAB, D2], mybir.dt.float32, kind="ExternalInput")
    idx16 = nc.dram_tensor("idx16", [128, tot // 16], mybir.dt.int16, kind="ExternalInput")
    # meta: [ramp(128) | lid(tiles) | val(tiles)]
    meta = nc.dram_tensor("meta", [128, 128 + 2 * tiles], mybir.dt.float32, kind="ExternalInput")
    out = nc.dram_tensor("out", [ROWS_PER_CORE, EMBED], mybir.dt.float32, kind="ExternalOutput")

    with TileContext(nc) as tc:
        with (
            tc.tile_pool(name="const", bufs=1) as cpool,
            tc.tile_pool(name="gath", bufs=28) as gpool,
            tc.tile_pool(name="w", bufs=96) as wpool,
            tc.tile_pool(name="psum", bufs=1, space="PSUM") as ppool,
            tc.tile_pool(name="dpsum", bufs=1, space="PSUM") as dpool,
            tc.tile_pool(name="outp", bufs=2) as opool,
        ):
            meta_sb = cpool.tile([128, 128 + 2 * tiles], mybir.dt.float32, tag="meta")
            nc.sync.dma_start(meta_sb[:, :], meta[:, :])
            ramp_sb = meta_sb[:, 0:128]
            lid_sb = meta_sb[:, 128:128 + tiles]
            val_sb = meta_sb[:, 128 + tiles:128 + 2 * tiles]
            idx_sb = cpool.tile([128, tot // 16], mybir.dt.int16, tag="idx")
            nc.sync.dma_start(idx_sb[:, :], idx16[:, :])

            psums = []
            for c in range(CHUNKS):
                ps = ppool.tile([128, D2], mybir.dt.float32, tag=f"ps{c}")
                psums.append(ps)
            dummy = dpool.tile([1, 1], mybir.dt.float32)

            GCAP = 1024  # HW limit on dma_gather num_idxs

            # Preload one Pool register per distinct num_idxs value. Passing
            # a raw int makes Bacc emit a MOVE right before every gather;
            # graph coloring reuses the same physical register, so each MOVE
            # carries a WAR on the in-flight gather's register read and
            # serializes the Pool queue (killing SWDGE context overlap).
            ng_all = []
            for b, (b0, n) in enumerate(bucket_span):
                for g0 in range(0, n, GCAP):
                    ng_all.append(min(GCAP, n - g0))
            ng_regs = {}
            for v in sorted(set(ng_all)):
                v = int(v)
                r = nc.gpsimd.alloc_register(f"ng_{v}")
                nc.gpsimd.reg_mov(r, v)
                ng_regs[v] = r

            call_id = 0
            for b, (b0, n) in enumerate(bucket_span):
                if n == 0:
                    continue
                tbl_slice = table[b * BUCKET:b * BUCKET + min(BUCKET, VOCAB - b * BUCKET), :]
                for g0 in range(0, n, GCAP):
                    ng = min(GCAP, n - g0)
                    e0 = b0 + g0            # absolute element offset
                    ntl = ng // 128
                    t0 = e0 // 128
                    rows = gpool.tile([128, GCAP // 128, D2], mybir.dt.float32, tag="rows")
                    with tc.high_priority(offset=200):
                        nc.gpsimd.dma_gather(
                            rows[:, 0:ntl, :],
                            tbl_slice,
                            idx_sb[:, e0 // 16:(e0 + ng) // 16],
                            ng, ng_regs[ng], D2, elem_step=D2,
                            queue_num=call_id % 4,
                        )
                    call_id += 1
                    # absorb the gather-DMA wait on PE so real matmuls carry one wait
                    nc.tensor.matmul(dummy[:, :], rows[0:1, 0, 0:1], rows[0:1, 0, 0:1],
                                     start=True, stop=True)
                    # q-half *= v for the whole call in ONE DVE op: in1 is the
                    # per-tile v vector broadcast along the 64 embed columns
                    # (stride-0 inner dim). Emitting it before the W builds
                    # keeps every matmul at a single DVE wait.
                    vap = val_sb[:, t0:t0 + ntl]
                    vbc = AP(vap.tensor, vap.offset,
                             [vap.ap[0], vap.ap[1], (0, EMBED)])
                    nc.vector.tensor_tensor(
                        rows[:, 0:ntl, EMBED:D2], rows[:, 0:ntl, EMBED:D2],
                        vbc, mybir.AluOpType.mult,
                    )
                    for tl in range(ntl):
                        t = t0 + tl
                        ch = int(tile_chunk[t])
                        wt = wpool.tile([128, 128], mybir.dt.float32, tag="wt")
                        nc.vector.tensor_scalar(
                            wt[:, :], ramp_sb,
                            lid_sb[:, t:t + 1], val_sb[:, t:t + 1],
                            mybir.AluOpType.is_equal, mybir.AluOpType.mult,
                        )
                        nc.tensor.matmul(
                            psums[ch][:, :], wt[:, :], rows[:, tl, :],
                            start=(t == first_tile[ch]), stop=(t == last_tile[ch]),
                            skip_group_check=True,
                        )

            for ch in range(CHUNKS):
                s_sb = opool.tile([128, EMBED], mybir.dt.float32, tag="s")
                nc.vector.tensor_copy(s_sb[:, :], psums[ch][:, 0:EMBED])
                o_sb = opool.tile([128, EMBED], mybir.dt.float32, tag="o")
                nc.vector.tensor_tensor(o_sb[:, :], s_sb[:, :], s_sb[:, :], mybir.AluOpType.mult)
                nc.vector.tensor_tensor(o_sb[:, :], o_sb[:, :], psums[ch][:, EMBED:D2], mybir.AluOpType.subtract)
                nc.vector.tensor_scalar(o_sb[:, :], o_sb[:, :], 0.5, None, mybir.AluOpType.mult)
                nc.sync.dma_start(out[ch * 128:(ch + 1) * 128, :], o_sb[:, :])

    _strip_redundant_self_waits(nc)
    nc.compile()
    return nc


_RUN_KWARGS = {}


def kernel(feature_embedding, feature_vals, batch_ids, feat_ids, batch_size):
    from concourse.bass_utils import run_bass_kernel_spmd

    feature_embedding = np.asarray(feature_embedding, dtype=np.float32)
    feature_vals = np.asarray(feature_vals, dtype=np.float32)
    batch_ids = np.asarray(batch_ids)
    feat_ids = np.asarray(feat_ids)

    table = np.concatenate([feature_embedding, feature_embedding * feature_embedding],
                           axis=1).astype(np.float32)

    (per_core, pad, seg_off, bucket_span, tot, tiles, tile_chunk,
     first_tile, last_tile) = _build_plan(feat_ids, batch_ids)

    in_maps = []
    for c in range(NCORES):
        idx16, lidf, valf = _build_core_arrays(
            per_core[c], pad, seg_off, bucket_span, tot, feature_vals)
        ramp = np.broadcast_to(np.arange(128, dtype=np.float32), (128, 128))
        meta = np.concatenate([ramp, lidf, valf], axis=1).astype(np.float32)
        in_maps.append({"table": table, "idx16": idx16, "meta": meta})

    nc = _build_bass(tot, tiles, bucket_span, tile_chunk, first_tile, last_tile)
    res = run_bass_kernel_spmd(nc, in_maps, core_ids=list(range(NCORES)), **_RUN_KWARGS)
    out = np.concatenate([res.results[c]["out"] for c in range(NCORES)], axis=0)
    if getattr(res, "exec_time_ns", None):
        kernel.last_exec_time_ns = res.exec_time_ns
    kernel.last_results = res
    return out.astype(np.float32)



# revision 8
# speedup vs baseline: 1.0430x; 1.0430x over previous
"""FM layer (gather + segment_sum + 0.5*(s^2 - sum_sq)) on 8 Trainium2 cores.

Strategy (data parallel over batch rows):
  - core c owns batch rows [512c, 512(c+1)); batch_ids are sorted, so each
    core's nonzeros are a contiguous slice found by searchsorted.
  - host packs the embedding table as [e | e^2] f32 -> 512B rows so one
    512B gather descriptor (line-rate on SDMA) delivers both factors.
  - per core, nonzeros are sorted by (vocab bucket of 32768 rows, 128-row
    batch chunk) and padded per (bucket,chunk) segment to a multiple of 128
    with (idx=0, v=0) entries; segment sizes are equalized across cores so
    a single SPMD program serves all 8 cores.
  - gpsimd.dma_gather calls of <=1024 rows (int16 indices into a 32768-row
    table slice; 1024 is a hard HW limit - 2048 faults the device) pull each
    bucket's rows to SBUF. Calls round-robin over 4 SWDGE queues with
    num_swdge_queues=4: desc-gen for different queues runs concurrently on
    the two SWDGE Q7 cores (~2x; measured 8.6us per 1024-idx gen, ~8.4ns/idx,
    which is the kernel's dominant cost - NOT the DMA bytes).
  - per gather call: one DVE tensor_tensor scales the whole call's e^2
    halves by v (per-tile v vector broadcast along the 64 embed cols via a
    stride-0 AP dim); then per 128-nonzero tile DVE builds
    W[p,b] = (ramp==lid[p])*v[p] and PE accumulates
    psum[chunk] += W^T @ [e | v*e^2] giving [s | q] per 128-row chunk.
    Emitting the scale before the W builds keeps each matmul at one DVE
    wait (avoids Bacc's event-semaphore split).
  - num_idxs registers are preloaded once per distinct gather size so no
    per-call MOVE sits in the Pool queue.
  - epilogue per chunk: out = 0.5*(s*s - q) on DVE, DMA to DRAM.
  - host concatenates the 8 per-core [512, 64] outputs.

History: baseline 1093us -> 810-818us (2-queue desc-gen overlap, batched
e^2 scale, deeper pools). Engine budget per core at 818us: Pool/Q7 desc-gen
~950us of instruction time (2 contexts overlap to ~6.8us/call effective),
DVE ~380us work, PE ~390us, SDMA ~340us busy.
"""
import numpy as np

NCORES = 8
VOCAB = 1000000
EMBED = 64
BATCH = 4096
ROWS_PER_CORE = BATCH // NCORES          # 512
CHUNKS = ROWS_PER_CORE // 128            # 4
BUCKET = 32768
NBUCKETS = (VOCAB + BUCKET - 1) // BUCKET  # 31 full + remainder => 31? 1e6/32768 = 30.5 -> 31
D2 = 2 * EMBED                           # 128 packed row elements


def _build_plan(feat_ids, batch_ids):
    """Host-side sharding plan. Returns per-core index/metadata arrays and the
    static tile schedule shared by all cores."""
    core_lo = np.searchsorted(batch_ids, np.arange(NCORES) * ROWS_PER_CORE)
    core_hi = np.searchsorted(batch_ids, (np.arange(NCORES) + 1) * ROWS_PER_CORE)

    per_core = []
    counts = np.zeros((NCORES, NBUCKETS, CHUNKS), np.int64)
    for c in range(NCORES):
        lo, hi = core_lo[c], core_hi[c]
        fid = feat_ids[lo:hi].astype(np.int64)
        bid = batch_ids[lo:hi].astype(np.int64) - c * ROWS_PER_CORE
        bucket = fid // BUCKET
        chunk = bid >> 7
        order = np.lexsort((chunk, bucket))
        fid, bid, bucket, chunk = fid[order], bid[order], bucket[order], chunk[order]
        per_core.append((lo, order, fid, bid, bucket, chunk))
        np.add.at(counts[c], (bucket, chunk), 1)

    pad = (np.ceil(counts.max(axis=0) / 128).astype(np.int64) * 128)  # [NBUCKETS, CHUNKS]
    seg_off = np.zeros((NBUCKETS, CHUNKS), np.int64)
    off = 0
    bucket_span = []  # (start_elem, n_elem) per bucket
    tile_chunk = []   # chunk id per global tile
    for b in range(NBUCKETS):
        b0 = off
        for ch in range(CHUNKS):
            seg_off[b, ch] = off
            tile_chunk.extend([ch] * (pad[b, ch] // 128))
            off += pad[b, ch]
        bucket_span.append((b0, off - b0))
    tot = off
    tiles = tot // 128
    tile_chunk = np.array(tile_chunk, np.int64)

    # first/last tile per chunk (for PSUM start/stop flags)
    first_tile = {}
    last_tile = {}
    for t, ch in enumerate(tile_chunk):
        first_tile.setdefault(int(ch), t)
        last_tile[int(ch)] = t

    return per_core, pad, seg_off, bucket_span, tot, tiles, tile_chunk, first_tile, last_tile


def _build_core_arrays(core_data, pad, seg_off, bucket_span, tot, feature_vals):
    lo, order, fid, bid, bucket, chunk = core_data
    v = feature_vals[lo:lo + len(order)].astype(np.float32)[order]

    # absolute position of every (sorted) nonzero in the padded stream;
    # within each (bucket, chunk) segment entries are consecutive in sorted order
    seg_id = bucket * pad.shape[1] + chunk
    seg_start = seg_off.reshape(-1)
    # index within segment = running count; sorted order makes segments contiguous
    change = np.ones(len(seg_id), bool)
    change[1:] = seg_id[1:] != seg_id[:-1]
    seg_first = np.where(change)[0]
    run = np.arange(len(seg_id)) - np.repeat(seg_first, np.diff(np.append(seg_first, len(seg_id))))
    pos = seg_start[seg_id] + run

    idx_local = np.zeros(tot, np.int64)          # index within 32768-row bucket slice
    lid = np.zeros(tot, np.float32)              # local row id within 128-row chunk
    val = np.zeros(tot, np.float32)
    idx_local[pos] = fid % BUCKET
    lid[pos] = (bid & 127).astype(np.float32)
    val[pos] = v

    # idx16: per-bucket-call wrap-16 layout, replicated across 8 groups of 16
    idx16 = np.zeros((128, tot // 16), np.int16)
    for b0, n in bucket_span:
        if n == 0:
            continue
        blk = idx_local[b0:b0 + n].reshape(n // 16, 16).T.astype(np.int16)  # [16, n/16]
        col0 = b0 // 16
        for g in range(8):
            idx16[g * 16:(g + 1) * 16, col0:col0 + n // 16] = blk

    tiles = tot // 128
    lidf = lid.reshape(tiles, 128).T.copy()      # [128, tiles]
    valf = val.reshape(tiles, 128).T.copy()
    return idx16, lidf, valf


def _strip_redundant_self_waits(nc):
    """Drop waits on an instruction's own engine-proc semaphore when program
    order already guarantees them (threshold <= prior same-engine updates).
    Tile emits these for pool-slot WAW; they force Bacc's one-wait-per-inst
    event-semaphore splitting, which serializes the hot loop."""
    import concourse.mybir as mybir

    proc_of_engine = {
        mybir.EngineType.PE: "PE_",
        mybir.EngineType.DVE: "DVE_",
        mybir.EngineType.Activation: "Activation_",
        mybir.EngineType.Pool: "Pool_",
        mybir.EngineType.SP: "SP_",
    }
    for blk in nc.m.functions[0].blocks:
        counts = {}
        for ins in blk.instructions:
            si = ins.sync_info
            eng = ins.engine
            pref = proc_of_engine.get(eng)
            if si is not None and si.on_wait and type(ins).__name__ == "InstDMAGatherAnt":
                # Keep only the PE slot-WAR leash and the idx-load (DMAHW)
                # wait. The DVE WAR is implied: each MM reads the q-scaled
                # rows, so MM(t) complete => q-scale(t) complete, and the PE
                # wait already covers the slot's MMs. The DMASW lane wait is
                # conservative bookkeeping (queue-0 DMAs are FIFO and sem
                # thresholds are monotonic). One wait avoids Bacc's
                # event-semaphore split, which serializes the pipeline.
                si.on_wait = [w for w in si.on_wait
                              if (w.ant_name or "").startswith(("PE_", "DMAHW"))]
            if si is not None and si.on_wait and pref is not None:
                kept = []
                for w in si.on_wait:
                    name = w.ant_name or ""
                    if (w.wait_mode == "sem-ge-imm"
                            and name.startswith(pref)
                            and w.wait_value <= counts.get(name, 0)):
                        continue
                    kept.append(w)
                if len(kept) != len(si.on_wait):
                    si.on_wait = kept
            if si is not None:
                for u in si.on_update:
                    name = u.ant_name or ""
                    if u.update_mode == "sem-inc" and name.startswith(
                            proc_of_engine.get(eng, "\0")):
                        counts[name] = counts.get(name, 0) + u.update_value


def _build_bass(tot, tiles, bucket_span, tile_chunk, first_tile, last_tile):
    import concourse.bacc as bacc
    import concourse.mybir as mybir
    from concourse.bass import AP
    from concourse.tile import TileContext

    # 2 SWDGE queues: desc-gen for alternating gathers runs on both SWDGE Q7
    # cores concurrently (measured 2x; 4 queues adds nothing - only 2 cores).
    nc = bacc.Bacc(trn_type="TRN2", num_swdge_queues=4)
    table = nc.dram_tensor("table", [VOCAB, D2], mybir.dt.float32, kind="ExternalInput")
    idx16 = nc.dram_tensor("idx16", [128, tot // 16], mybir.dt.int16, kind="ExternalInput")
    # meta: [ramp(128) | lid(tiles) | val(tiles)]
    meta = nc.dram_tensor("meta", [128, 128 + 2 * tiles], mybir.dt.float32, kind="ExternalInput")
    out = nc.dram_tensor("out", [ROWS_PER_CORE, EMBED], mybir.dt.float32, kind="ExternalOutput")

    with TileContext(nc) as tc:
        with (
            tc.tile_pool(name="const", bufs=1) as cpool,
            tc.tile_pool(name="gath", bufs=28) as gpool,
            tc.tile_pool(name="w", bufs=96) as wpool,
            tc.tile_pool(name="psum", bufs=1, space="PSUM") as ppool,
            tc.tile_pool(name="dpsum", bufs=1, space="PSUM") as dpool,
            tc.tile_pool(name="outp", bufs=2) as opool,
        ):
            meta_sb = cpool.tile([128, 128 + 2 * tiles], mybir.dt.float32, tag="meta")
            nc.sync.dma_start(meta_sb[:, :], meta[:, :])
            ramp_sb = meta_sb[:, 0:128]
            lid_sb = meta_sb[:, 128:128 + tiles]
            val_sb = meta_sb[:, 128 + tiles:128 + 2 * tiles]
            idx_sb = cpool.tile([128, tot // 16], mybir.dt.int16, tag="idx")
            nc.sync.dma_start(idx_sb[:, :], idx16[:, :])

            psums = []
            for c in range(CHUNKS):
                ps = ppool.tile([128, D2], mybir.dt.float32, tag=f"ps{c}")
                psums.append(ps)
            dummy = dpool.tile([1, 1], mybir.dt.float32)

            GCAP = 1024  # HW limit on dma_gather num_idxs

            # Preload one Pool register per distinct num_idxs value. Passing
            # a raw int makes Bacc emit a MOVE right before every gather;
            # graph coloring reuses the same physical register, so each MOVE
            # carries a WAR on the in-flight gather's register read and
            # serializes the Pool queue (killing SWDGE context overlap).
            ng_all = []
            for b, (b0, n) in enumerate(bucket_span):
                for g0 in range(0, n, GCAP):
                    ng_all.append(min(GCAP, n - g0))
            ng_regs = {}
            for v in sorted(set(ng_all)):
                v = int(v)
                r = nc.gpsimd.alloc_register(f"ng_{v}")
                nc.gpsimd.reg_mov(r, v)
                ng_regs[v] = r

            call_id = 0
            for b, (b0, n) in enumerate(bucket_span):
                if n == 0:
                    continue
                tbl_slice = table[b * BUCKET:b * BUCKET + min(BUCKET, VOCAB - b * BUCKET), :]
                for g0 in range(0, n, GCAP):
                    ng = min(GCAP, n - g0)
                    e0 = b0 + g0            # absolute element offset
                    ntl = ng // 128
                    t0 = e0 // 128
                    rows = gpool.tile([128, GCAP // 128, D2], mybir.dt.float32, tag="rows")
                    with tc.high_priority(offset=200):
                        nc.gpsimd.dma_gather(
                            rows[:, 0:ntl, :],
                            tbl_slice,
                            idx_sb[:, e0 // 16:(e0 + ng) // 16],
                            ng, ng_regs[ng], D2, elem_step=D2,
                            queue_num=call_id % 4,
                        )
                    call_id += 1
                    # absorb the gather-DMA wait on PE so real matmuls carry one wait
                    nc.tensor.matmul(dummy[:, :], rows[0:1, 0, 0:1], rows[0:1, 0, 0:1],
                                     start=True, stop=True)
                    # q-half *= v for the whole call in ONE DVE op: in1 is the
                    # per-tile v vector broadcast along the 64 embed columns
                    # (stride-0 inner dim). Emitting it before the W builds
                    # keeps every matmul at a single DVE wait.
                    vap = val_sb[:, t0:t0 + ntl]
                    vbc = AP(vap.tensor, vap.offset,
                             [vap.ap[0], vap.ap[1], (0, EMBED)])
                    nc.vector.tensor_tensor(
                        rows[:, 0:ntl, EMBED:D2], rows[:, 0:ntl, EMBED:D2],
                        vbc, mybir.AluOpType.mult,
                    )
                    for tl in range(ntl):
                        t = t0 + tl
                        ch = int(tile_chunk[t])
                        wt = wpool.tile([128, 128], mybir.dt.float32, tag="wt")
                        nc.vector.tensor_scalar(
                            wt[:, :], ramp_sb,
                            lid_sb[:, t:t + 1], val_sb[:, t:t + 1],
                            mybir.AluOpType.is_equal, mybir.AluOpType.mult,
                        )
                        nc.tensor.matmul(
                            psums[ch][:, :], wt[:, :], rows[:, tl, :],
                            start=(t == first_tile[ch]), stop=(t == last_tile[ch]),
                            skip_group_check=True,
                        )

            for ch in range(CHUNKS):
                s_sb = opool.tile([128, EMBED], mybir.dt.float32, tag="s")
                nc.vector.tensor_copy(s_sb[:, :], psums[ch][:, 0:EMBED])
                o_sb = opool.tile([128, EMBED], mybir.dt.float32, tag="o")
                nc.vector.tensor_tensor(o_sb[:, :], s_sb[:, :], s_sb[:, :], mybir.AluOpType.mult)
                nc.vector.tensor_tensor(o_sb[:, :], o_sb[:, :], psums[ch][:, EMBED:D2], mybir.AluOpType.subtract)
                nc.vector.tensor_scalar(o_sb[:, :], o_sb[:, :], 0.5, None, mybir.AluOpType.mult)
                nc.sync.dma_start(out[ch * 128:(ch + 1) * 128, :], o_sb[:, :])

    _strip_redundant_self_waits(nc)
    nc.compile()
    return nc


_RUN_KWARGS = {}


def kernel(feature_embedding, feature_vals, batch_ids, feat_ids, batch_size):
    from concourse.bass_utils import run_bass_kernel_spmd

    feature_embedding = np.asarray(feature_embedding, dtype=np.float32)
    feature_vals = np.asarray(feature_vals, dtype=np.float32)
    batch_ids = np.asarray(batch_ids)
    feat_ids = np.asarray(feat_ids)

    table = np.concatenate([feature_embedding, feature_embedding * feature_embedding],
                           axis=1).astype(np.float32)

    (per_core, pad, seg_off, bucket_span, tot, tiles, tile_chunk,
     first_tile, last_tile) = _build_plan(feat_ids, batch_ids)

    in_maps = []
    for c in range(NCORES):
        idx16, lidf, valf = _build_core_arrays(
            per_core[c], pad, seg_off, bucket_span, tot, feature_vals)
        ramp = np.broadcast_to(np.arange(128, dtype=np.float32), (128, 128))
        meta = np.concatenate([ramp, lidf, valf], axis=1).astype(np.float32)
        in_maps.append({"table": table, "idx16": idx16, "meta": meta})

    nc = _build_bass(tot, tiles, bucket_span, tile_chunk, first_tile, last_tile)
    res = run_bass_kernel_spmd(nc, in_maps, core_ids=list(range(NCORES)), **_RUN_KWARGS)
    out = np.concatenate([res.results[c]["out"] for c in range(NCORES)], axis=0)
    if getattr(res, "exec_time_ns", None):
        kernel.last_exec_time_ns = res.exec_time_ns
    kernel.last_results = res
    return out.astype(np.float32)

